# revision 9
# baseline (speedup 1.0000x reference)
"""Sparse attention (B=4,H=16,N=2048,D=64) on 8 trn2 NeuronCores.

Sharding: core c = bp*4 + hq handles batches [2bp, 2bp+1] x heads [4hq..4hq+3].
Device computes, per (b,h):  O^T = normalize( V~^T @ (mask^T * exp(K Q^T/8 + bias^T)) )
with V~ = [V | 1] so the softmax denominator falls out of the same matmul.
Host does layout transforms (transposes/casts) and the final gather.
"""

import numpy as np
import ml_dtypes

import concourse.bass as bass
from concourse import bacc
import concourse.mybir as mybir
import concourse.tile as tile
from concourse.bass_utils import run_bass_kernel_spmd

dt = mybir.dt
AF = mybir.ActivationFunctionType

B, H, N, D = 4, 16, 2048, 64
NB = 2   # batches per core
NH = 4   # heads per core
P = 128
NKT = N // P          # 16 key tiles
QW = 512              # query tile width (one PSUM bank of fp32)
SUPW = 1024           # S supertile width (2 banks) -> fewer/larger ACT ops
NQP = N // SUPW       # 2 query supertiles
TRACE = False

_CACHE = {}


def build_bass():
    nc = bacc.Bacc()
    qT = nc.declare_dram_parameter("qT", [NB, NH, D, N], dt.float32r, isOutput=False)
    kT = nc.declare_dram_parameter("kT", [NB, NH, D, N], dt.float32r, isOutput=False)
    vA = nc.declare_dram_parameter("vA", [NB, NH, N, D + 1], dt.float16, isOutput=False)
    maskT = nc.declare_dram_parameter("maskT", [NB, N, N], dt.float16, isOutput=False)
    biasT = nc.declare_dram_parameter("biasT", [NH, N, N], dt.float16, isOutput=False)
    ident = nc.declare_dram_parameter("ident", [P, P], dt.float16, isOutput=False)
    ones = nc.declare_dram_parameter("ones", [1, D], dt.float16, isOutput=False)
    outT = nc.declare_dram_parameter("outT", [NB, NH, D, N], dt.float32, isOutput=True)

    with tile.TileContext(nc) as tc:
        with (
            tc.tile_pool(name="const", bufs=1) as cpool,
            tc.tile_pool(name="mask", bufs=1) as mpool,
            tc.tile_pool(name="qk", bufs=1) as qkpool,
            tc.tile_pool(name="vp", bufs=1) as vpool,
            tc.tile_pool(name="bias", bufs=3) as bpool,
            tc.tile_pool(name="pt", bufs=3) as ppool,
            tc.tile_pool(name="norm", bufs=2) as rpool,
            tc.tile_pool(name="out", bufs=3) as opool_sb,
            tc.tile_pool(name="spsum", bufs=2, space="PSUM") as spool,
            tc.tile_pool(name="opsum", bufs=1, space="PSUM") as opool,
        ):
            ident_sb = cpool.tile([P, P], dt.float16, tag="ident")
            nc.sync.dma_start(ident_sb, ident[:])
            ones_sb = cpool.tile([1, D], dt.float16, tag="ones")
            nc.sync.dma_start(ones_sb, ones[:])

            # resident transposed masks (fp16 0/1), one per batch
            mask_sb = []
            for b in range(NB):
                m = mpool.tile([P, NKT, N], dt.float16, tag=f"mask{b}")
                nc.sync.dma_start(m, maskT[b].rearrange("(t p) i -> p t i", p=P))
                mask_sb.append(m)

            for h in range(NH):
                qsb, ksb, vsb = [], [], []
                for b in range(NB):
                    qt_ = qkpool.tile([D, N], dt.float32r, tag=f"q{b}")
                    nc.sync.dma_start(qt_, qT[b, h])
                    kt_ = qkpool.tile([D, N], dt.float32r, tag=f"k{b}")
                    nc.sync.dma_start(kt_, kT[b, h])
                    vt_ = vpool.tile([P, NKT, D + 1], dt.float16, tag=f"v{b}")
                    nc.sync.dma_start(vt_, vA[b, h].rearrange("(t p) c -> p t c", p=P))
                    qsb.append(qt_)
                    ksb.append(kt_)
                    vsb.append(vt_)

                for qp in range(NQP):
                    q0 = qp * SUPW
                    opsum = []
                    for b in range(NB):
                        row = []
                        for qi in range(SUPW // QW):
                            ot = opool.tile([D + 1, QW], dt.float32,
                                            tag=f"o{b}{qi}", name=f"opsum{b}{qi}")
                            row.append(ot)
                        opsum.append(row)
                    for kt in range(NKT):
                        bias_sb = bpool.tile([P, SUPW], dt.float16, tag="bias")
                        nc.sync.dma_start(
                            bias_sb,
                            biasT[h, kt * P:(kt + 1) * P, q0:q0 + SUPW],
                        )
                        for b in range(NB):
                            ssup = spool.tile([P, SUPW], dt.float32, tag="s")
                            for qi in range(SUPW // QW):
                                nc.tensor.matmul(
                                    ssup[:, qi * QW:(qi + 1) * QW],
                                    ksb[b][:, kt * P:(kt + 1) * P],
                                    qsb[b][:, q0 + qi * QW:q0 + (qi + 1) * QW],
                                    start=True, stop=False,
                                )
                            for qi in range(SUPW // QW):
                                nc.tensor.matmul(
                                    ssup[:, qi * QW:(qi + 1) * QW],
                                    ident_sb,
                                    bias_sb[:, qi * QW:(qi + 1) * QW],
                                    start=False, stop=True,
                                )
                            pt = ppool.tile([P, SUPW], dt.float16, tag="pt")
                            nc.scalar.activation(pt, ssup, AF.Exp)
                            nc.vector.tensor_mul(
                                pt, pt, mask_sb[b][:, kt, q0:q0 + SUPW]
                            )
                            for qi in range(SUPW // QW):
                                nc.tensor.matmul(
                                    opsum[b][qi],
                                    vsb[b][:, kt, :],
                                    pt[:, qi * QW:(qi + 1) * QW],
                                    start=(kt == 0), stop=(kt == NKT - 1),
                                )
                    for b in range(NB):
                        for qi in range(SUPW // QW):
                            recip = rpool.tile([1, QW], dt.float16, tag="recip")
                            with nc.allow_low_precision(
                                reason="softmax denom recip in fp16 (values ~2e-4)"
                            ):
                                nc.vector.reciprocal(
                                    recip, opsum[b][qi][D:D + 1, :]
                                )
                            bc = spool.tile([D, QW], dt.float32, tag="s")
                            nc.tensor.matmul(
                                bc, ones_sb, recip,
                                start=True, stop=True,
                            )
                            rec64 = rpool.tile([D, QW], dt.float32, tag="rec64")
                            nc.vector.tensor_copy(rec64, bc)
                            osb = opool_sb.tile([D, QW], dt.float32, tag="osb")
                            nc.vector.tensor_mul(osb, opsum[b][qi][:D, :], rec64)
                            nc.sync.dma_start(
                                outT[b, h, :, q0 + qi * QW:q0 + (qi + 1) * QW], osb
                            )
    nc.finalize()
    return nc


def make_in_maps(q, k, v, mask, attn_bias):
    scale = np.float32(D ** -0.5)
    qT = np.ascontiguousarray(q.transpose(0, 1, 3, 2)) * scale   # [B,H,D,N] f32
    kT = np.ascontiguousarray(k.transpose(0, 1, 3, 2))
    vA = np.concatenate(
        [v, np.ones((B, H, N, 1), np.float32)], axis=-1
    ).astype(np.float16)                                         # [B,H,N,D+1]
    maskT = np.ascontiguousarray(
        mask[:, 0].transpose(0, 2, 1)
    ).astype(np.float16)                                         # [B,N,N] 0/1
    biasT = np.ascontiguousarray(
        attn_bias[0].transpose(0, 2, 1)
    ).astype(np.float16)                                         # [H,N,N]
    ident = np.eye(P, dtype=np.float16)
    ones = np.ones((1, D), np.float16)

    in_maps = []
    for c in range(8):
        bp, hq = divmod(c, 4)
        bs, hs = 2 * bp, 4 * hq
        in_maps.append({
            "qT": np.ascontiguousarray(qT[bs:bs + NB, hs:hs + NH]),
            "kT": np.ascontiguousarray(kT[bs:bs + NB, hs:hs + NH]),
            "vA": np.ascontiguousarray(vA[bs:bs + NB, hs:hs + NH]),
            "maskT": np.ascontiguousarray(maskT[bs:bs + NB]),
            "biasT": np.ascontiguousarray(biasT[hs:hs + NH]),
            "ident": ident,
            "ones": ones,
        })
    return in_maps


def kernel(q, k, v, mask, attn_bias):
    if "nc" not in _CACHE:
        _CACHE["nc"] = build_bass()
    nc = _CACHE["nc"]
    in_maps = make_in_maps(
        np.asarray(q, np.float32), np.asarray(k, np.float32),
        np.asarray(v, np.float32), np.asarray(mask, bool),
        np.asarray(attn_bias, np.float32),
    )
    rr = run_bass_kernel_spmd(
        nc, in_maps, list(range(8)), trace=TRACE,
        tmpdir=_CACHE.get("tmpdir"),
    )
    _CACHE["last_result"] = rr

    out = np.empty((B, H, N, D), np.float32)
    for c in range(8):
        bp, hq = divmod(c, 4)
        bs, hs = 2 * bp, 4 * hq
        oT = np.asarray(rr.results[c]["outT"])    # [NB,NH,D,N]
        out[bs:bs + NB, hs:hs + NH] = oT.transpose(0, 1, 3, 2)
    return out


# revision 12
# speedup vs baseline: 1.2214x; 1.2214x over previous
"""Sparse attention (B=4,H=16,N=2048,D=64) on 8 trn2 NeuronCores.

Sharding: core c = bp*4 + hq handles batches [2bp, 2bp+1] x heads [4hq..4hq+3].
Device computes, per (b,h):  O^T = normalize( V~^T @ (mask^T * exp(K Q^T/8 + bias^T)) )
with V~ = [V | 1] so the softmax denominator falls out of the same matmul.
The two batches' S-matmuls (K=64 each) run concurrently in PE row-groups
0-63 / 64-127 via tile_position. Host does layout transforms and the gather.
"""

import numpy as np
import ml_dtypes

import concourse.bass as bass
from concourse import bacc
import concourse.mybir as mybir
import concourse.tile as tile
from concourse.bass_utils import run_bass_kernel_spmd

dt = mybir.dt
AF = mybir.ActivationFunctionType

B, H, N, D = 4, 16, 2048, 64
NB = 2   # batches per core
NH = 4   # heads per core
P = 128
NKT = N // P          # 16 key tiles
QW = 512              # query tile width (one PSUM bank of fp32)
SUPW = 1024           # S supertile width (2 banks) -> fewer/larger ACT ops
NQP = N // SUPW       # 2 query supertiles
NQI = SUPW // QW      # 2 PSUM-bank columns per supertile
TRACE = False

_CACHE = {}


def build_bass():
    nc = bacc.Bacc()
    # q/k packed per head: partitions 0-63 = batch 0 (d-dim), 64-127 = batch 1
    qT = nc.declare_dram_parameter("qT", [NH, 2 * D, N], dt.float16, isOutput=False)
    kT = nc.declare_dram_parameter("kT", [NH, 2 * D, N], dt.float16, isOutput=False)
    vA = nc.declare_dram_parameter("vA", [NB, NH, N, D + 1], dt.float16, isOutput=False)
    maskT = nc.declare_dram_parameter("maskT", [NB, N, N], dt.float16, isOutput=False)
    biasT = nc.declare_dram_parameter("biasT", [NH, N, N], dt.float16, isOutput=False)
    ident = nc.declare_dram_parameter("ident", [P, P], dt.float16, isOutput=False)
    ones = nc.declare_dram_parameter("ones", [1, D], dt.float16, isOutput=False)
    outT = nc.declare_dram_parameter("outT", [NB, NH, D, N], dt.float32, isOutput=True)

    with tile.TileContext(nc) as tc:
        with (
            tc.tile_pool(name="const", bufs=1) as cpool,
            tc.tile_pool(name="mask", bufs=1) as mpool,
            tc.tile_pool(name="qk", bufs=2) as qkpool,
            tc.tile_pool(name="vp", bufs=2) as vpool,
            tc.tile_pool(name="bias", bufs=3) as bpool,
            tc.tile_pool(name="pt", bufs=3) as ppool,
            tc.tile_pool(name="norm", bufs=1) as rpool,
            tc.tile_pool(name="out", bufs=3) as opool_sb,
            tc.tile_pool(name="spsum", bufs=2, space="PSUM") as spool,
            tc.tile_pool(name="opsum", bufs=1, space="PSUM") as opool,
        ):
            ident_sb = cpool.tile([P, P], dt.float16, tag="ident")
            nc.sync.dma_start(ident_sb, ident[:])
            ones_sb = cpool.tile([1, D], dt.float16, tag="ones")
            nc.sync.dma_start(ones_sb, ones[:])

            # resident transposed masks (fp16 0/1), one per batch
            mask_sb = []
            for b in range(NB):
                m = mpool.tile([P, NKT, N], dt.float16, tag=f"mask{b}")
                nc.sync.dma_start(m, maskT[b].rearrange("(t p) i -> p t i", p=P))
                mask_sb.append(m)

            for h in range(NH):
                qsb = qkpool.tile([2 * D, N], dt.float16, tag="q")
                nc.sync.dma_start(qsb, qT[h])
                ksb = qkpool.tile([2 * D, N], dt.float16, tag="k")
                nc.sync.dma_start(ksb, kT[h])
                vsb = []
                for b in range(NB):
                    vt_ = vpool.tile([P, NKT, D + 1], dt.float16, tag=f"v{b}")
                    nc.sync.dma_start(vt_, vA[b, h].rearrange("(t p) c -> p t c", p=P))
                    vsb.append(vt_)

                for qp in range(NQP):
                    q0 = qp * SUPW
                    opsum = []
                    for b in range(NB):
                        row = []
                        for qi in range(NQI):
                            ot = opool.tile([D + 1, QW], dt.float32,
                                            tag=f"o{b}{qi}", name=f"opsum{b}{qi}")
                            row.append(ot)
                        opsum.append(row)
                    for kt in range(NKT):
                        bias_sb = bpool.tile([P, SUPW], dt.float16, tag="bias")
                        nc.sync.dma_start(
                            bias_sb,
                            biasT[h, kt * P:(kt + 1) * P, q0:q0 + SUPW],
                        )
                        ssup = [spool.tile([P, SUPW], dt.float32, tag="s",
                                           name=f"ssup{b}")
                                for b in range(NB)]
                        # batch-packed S matmuls: b=0 in PE rows 0-63,
                        # b=1 in rows 64-127 -> run concurrently
                        for qi in range(NQI):
                            for b in range(NB):
                                nc.tensor.matmul(
                                    ssup[b][:, qi * QW:(qi + 1) * QW],
                                    ksb[b * D:(b + 1) * D, kt * P:(kt + 1) * P],
                                    qsb[b * D:(b + 1) * D,
                                        q0 + qi * QW:q0 + (qi + 1) * QW],
                                    start=True, stop=False,
                                    tile_position=(b * D, 0),
                                )
                        for b in range(NB):
                            for qi in range(NQI):
                                nc.tensor.matmul(
                                    ssup[b][:, qi * QW:(qi + 1) * QW],
                                    ident_sb,
                                    bias_sb[:, qi * QW:(qi + 1) * QW],
                                    start=False, stop=True,
                                )
                            pt = ppool.tile([P, SUPW], dt.float16, tag="pt")
                            nc.scalar.activation(pt, ssup[b], AF.Exp)
                            nc.vector.tensor_mul(
                                pt, pt, mask_sb[b][:, kt, q0:q0 + SUPW]
                            )
                            for qi in range(NQI):
                                nc.tensor.matmul(
                                    opsum[b][qi],
                                    vsb[b][:, kt, :],
                                    pt[:, qi * QW:(qi + 1) * QW],
                                    start=(kt == 0), stop=(kt == NKT - 1),
                                )
                    # normalize: batch the 4 denominator rows -> one approx recip
                    sums = rpool.tile([1, NB * NQI * QW], dt.float32, tag="sums")
                    for b in range(NB):
                        for qi in range(NQI):
                            j = b * NQI + qi
                            nc.vector.tensor_copy(
                                sums[:, j * QW:(j + 1) * QW],
                                opsum[b][qi][D:D + 1, :],
                            )
                    rec = rpool.tile([1, NB * NQI * QW], dt.float32, tag="rec")
                    nc.vector.reciprocal_approx_fast(rec, sums)
                    rec16 = rpool.tile([1, NB * NQI * QW], dt.float16, tag="rec16")
                    nc.vector.tensor_copy(rec16, rec)
                    for b in range(NB):
                        for qi in range(NQI):
                            j = b * NQI + qi
                            bc = spool.tile([D, QW], dt.float32, tag="s")
                            nc.tensor.matmul(
                                bc, ones_sb, rec16[:, j * QW:(j + 1) * QW],
                                start=True, stop=True,
                            )
                            rec64 = rpool.tile([D, QW], dt.float32, tag="rec64")
                            nc.vector.tensor_copy(rec64, bc)
                            osb = opool_sb.tile([D, QW], dt.float32, tag="osb")
                            nc.vector.tensor_mul(osb, opsum[b][qi][:D, :], rec64)
                            nc.sync.dma_start(
                                outT[b, h, :, q0 + qi * QW:q0 + (qi + 1) * QW], osb
                            )
    nc.finalize()
    return nc


def make_in_maps(q, k, v, mask, attn_bias):
    scale = np.float32(D ** -0.5)
    qTf = (q.transpose(0, 1, 3, 2) * scale).astype(np.float16)   # [B,H,D,N]
    kTf = k.transpose(0, 1, 3, 2).astype(np.float16)
    vA = np.concatenate(
        [v, np.ones((B, H, N, 1), np.float32)], axis=-1
    ).astype(np.float16)                                         # [B,H,N,D+1]
    maskT = np.ascontiguousarray(
        mask[:, 0].transpose(0, 2, 1)
    ).astype(np.float16)                                         # [B,N,N] 0/1
    biasT = np.ascontiguousarray(
        attn_bias[0].transpose(0, 2, 1)
    ).astype(np.float16)                                         # [H,N,N]
    ident = np.eye(P, dtype=np.float16)
    ones = np.ones((1, D), np.float16)

    in_maps = []
    for c in range(8):
        bp, hq = divmod(c, 4)
        bs, hs = 2 * bp, 4 * hq
        # pack the two batches along the partition dim: [NH, 2D, N]
        qpack = np.concatenate(
            [qTf[bs + b, hs:hs + NH] for b in range(NB)], axis=1
        )  # [NH, 2D, N]
        kpack = np.concatenate(
            [kTf[bs + b, hs:hs + NH] for b in range(NB)], axis=1
        )
        in_maps.append({
            "qT": np.ascontiguousarray(qpack),
            "kT": np.ascontiguousarray(kpack),
            "vA": np.ascontiguousarray(vA[bs:bs + NB, hs:hs + NH]),
            "maskT": np.ascontiguousarray(maskT[bs:bs + NB]),
            "biasT": np.ascontiguousarray(biasT[hs:hs + NH]),
            "ident": ident,
            "ones": ones,
        })
    return in_maps


def kernel(q, k, v, mask, attn_bias):
    if "nc" not in _CACHE:
        _CACHE["nc"] = build_bass()
    nc = _CACHE["nc"]
    in_maps = make_in_maps(
        np.asarray(q, np.float32), np.asarray(k, np.float32),
        np.asarray(v, np.float32), np.asarray(mask, bool),
        np.asarray(attn_bias, np.float32),
    )
    rr = run_bass_kernel_spmd(
        nc, in_maps, list(range(8)), trace=TRACE,
        tmpdir=_CACHE.get("tmpdir"),
    )
    _CACHE["last_result"] = rr

    out = np.empty((B, H, N, D), np.float32)
    for c in range(8):
        bp, hq = divmod(c, 4)
        bs, hs = 2 * bp, 4 * hq
        oT = np.asarray(rr.results[c]["outT"])    # [NB,NH,D,N]
        out[bs:bs + NB, hs:hs + NH] = oT.transpose(0, 1, 3, 2)
    return out


# revision 13
# speedup vs baseline: 1.2278x; 1.0052x over previous
"""Sparse attention (B=4,H=16,N=2048,D=64) on 8 trn2 NeuronCores.

Sharding: core c = bp*4 + hq handles batches [2bp, 2bp+1] x heads [4hq..4hq+3].
Device computes, per (b,h):  O^T = normalize( V~^T @ (mask^T * exp(K Q^T/8 + bias^T)) )
with V~ = [V | 1] so the softmax denominator falls out of the same matmul.
The two batches' S-matmuls (K=64 each) run concurrently in PE row-groups
0-63 / 64-127 via tile_position. Host does layout transforms and the gather.
"""

import numpy as np
import ml_dtypes

import concourse.bass as bass
from concourse import bacc
import concourse.mybir as mybir
import concourse.tile as tile
from concourse.bass_utils import run_bass_kernel_spmd

dt = mybir.dt
AF = mybir.ActivationFunctionType

B, H, N, D = 4, 16, 2048, 64
NB = 2   # batches per core
NH = 4   # heads per core
P = 128
NKT = N // P          # 16 key tiles
QW = 512              # query tile width (one PSUM bank of fp32)
SUPW = 1024           # S supertile width (2 banks) -> fewer/larger ACT ops
NQP = N // SUPW       # 2 query supertiles
NQI = SUPW // QW      # 2 PSUM-bank columns per supertile
TRACE = False

_CACHE = {}


def build_bass():
    nc = bacc.Bacc()
    # q/k packed per head: partitions 0-63 = batch 0 (d-dim), 64-127 = batch 1
    qT = nc.declare_dram_parameter("qT", [NH, 2 * D, N], dt.float16, isOutput=False)
    kT = nc.declare_dram_parameter("kT", [NH, 2 * D, N], dt.float16, isOutput=False)
    vA = nc.declare_dram_parameter("vA", [NB, NH, N, D + 1], dt.float16, isOutput=False)
    maskT = nc.declare_dram_parameter("maskT", [1, N, N], dt.float16, isOutput=False)
    mex1 = nc.declare_dram_parameter("mex1", [NH, N, N], dt.float16, isOutput=False)
    biasT = nc.declare_dram_parameter("biasT", [NH, N, N], dt.float16, isOutput=False)
    ident = nc.declare_dram_parameter("ident", [P, P], dt.float16, isOutput=False)
    ones = nc.declare_dram_parameter("ones", [1, D], dt.float16, isOutput=False)
    outT = nc.declare_dram_parameter("outT", [NB, NH, D, N], dt.float32, isOutput=True)

    with tile.TileContext(nc) as tc:
        with (
            tc.tile_pool(name="const", bufs=1) as cpool,
            tc.tile_pool(name="mask", bufs=1) as mpool,
            tc.tile_pool(name="qk", bufs=2) as qkpool,
            tc.tile_pool(name="vp", bufs=2) as vpool,
            tc.tile_pool(name="bias", bufs=3) as bpool,
            tc.tile_pool(name="pt", bufs=3) as ppool,
            tc.tile_pool(name="norm", bufs=1) as rpool,
            tc.tile_pool(name="out", bufs=3) as opool_sb,
            tc.tile_pool(name="spsum", bufs=2, space="PSUM") as spool,
            tc.tile_pool(name="opsum", bufs=1, space="PSUM") as opool,
        ):
            ident_sb = cpool.tile([P, P], dt.float16, tag="ident")
            nc.sync.dma_start(ident_sb, ident[:])
            ones_sb = cpool.tile([1, D], dt.float16, tag="ones")
            nc.sync.dma_start(ones_sb, ones[:])

            # resident transposed mask (fp16 0/1) for batch 0 only;
            # batch 1 uses streamed mask*exp(bias) tiles instead
            mask0 = mpool.tile([P, NKT, N], dt.float16, tag="mask0")
            nc.sync.dma_start(mask0, maskT[0].rearrange("(t p) i -> p t i", p=P))

            for h in range(NH):
                qsb = qkpool.tile([2 * D, N], dt.float16, tag="q")
                nc.sync.dma_start(qsb, qT[h])
                ksb = qkpool.tile([2 * D, N], dt.float16, tag="k")
                nc.sync.dma_start(ksb, kT[h])
                vsb = []
                for b in range(NB):
                    vt_ = vpool.tile([P, NKT, D + 1], dt.float16, tag=f"v{b}")
                    nc.sync.dma_start(vt_, vA[b, h].rearrange("(t p) c -> p t c", p=P))
                    vsb.append(vt_)

                for qp in range(NQP):
                    q0 = qp * SUPW
                    opsum = []
                    for b in range(NB):
                        row = []
                        for qi in range(NQI):
                            ot = opool.tile([D + 1, QW], dt.float32,
                                            tag=f"o{b}{qi}", name=f"opsum{b}{qi}")
                            row.append(ot)
                        opsum.append(row)
                    for kt in range(NKT):
                        bias_sb = bpool.tile([P, SUPW], dt.float16, tag="bias")
                        nc.sync.dma_start(
                            bias_sb,
                            biasT[h, kt * P:(kt + 1) * P, q0:q0 + SUPW],
                        )
                        ssup = [spool.tile([P, SUPW], dt.float32, tag="s",
                                           name=f"ssup{b}")
                                for b in range(NB)]
                        # batch-packed S matmuls: b=0 in PE rows 0-63,
                        # b=1 in rows 64-127 -> run concurrently
                        for qi in range(NQI):
                            for b in range(NB):
                                nc.tensor.matmul(
                                    ssup[b][:, qi * QW:(qi + 1) * QW],
                                    ksb[b * D:(b + 1) * D, kt * P:(kt + 1) * P],
                                    qsb[b * D:(b + 1) * D,
                                        q0 + qi * QW:q0 + (qi + 1) * QW],
                                    start=True, stop=(b == 1),
                                    tile_position=(b * D, 0),
                                )
                        mex_sb = bpool.tile([P, SUPW], dt.float16, tag="mex")
                        nc.sync.dma_start(
                            mex_sb,
                            mex1[h, kt * P:(kt + 1) * P, q0:q0 + SUPW],
                        )
                        for b in range(NB):
                            if b == 0:
                                for qi in range(NQI):
                                    nc.tensor.matmul(
                                        ssup[b][:, qi * QW:(qi + 1) * QW],
                                        ident_sb,
                                        bias_sb[:, qi * QW:(qi + 1) * QW],
                                        start=False, stop=True,
                                    )
                            pt = ppool.tile([P, SUPW], dt.float16, tag="pt")
                            nc.scalar.activation(pt, ssup[b], AF.Exp)
                            nc.vector.tensor_mul(
                                pt, pt,
                                mask0[:, kt, q0:q0 + SUPW] if b == 0
                                else mex_sb,
                            )
                            for qi in range(NQI):
                                nc.tensor.matmul(
                                    opsum[b][qi],
                                    vsb[b][:, kt, :],
                                    pt[:, qi * QW:(qi + 1) * QW],
                                    start=(kt == 0), stop=(kt == NKT - 1),
                                )
                    # normalize: batch the 4 denominator rows -> one approx recip
                    sums = rpool.tile([1, NB * NQI * QW], dt.float32, tag="sums")
                    for b in range(NB):
                        for qi in range(NQI):
                            j = b * NQI + qi
                            nc.vector.tensor_copy(
                                sums[:, j * QW:(j + 1) * QW],
                                opsum[b][qi][D:D + 1, :],
                            )
                    rec = rpool.tile([1, NB * NQI * QW], dt.float32, tag="rec")
                    nc.vector.reciprocal_approx_fast(rec, sums)
                    rec16 = rpool.tile([1, NB * NQI * QW], dt.float16, tag="rec16")
                    nc.vector.tensor_copy(rec16, rec)
                    for b in range(NB):
                        for qi in range(NQI):
                            j = b * NQI + qi
                            bc = spool.tile([D, QW], dt.float32, tag="s")
                            nc.tensor.matmul(
                                bc, ones_sb, rec16[:, j * QW:(j + 1) * QW],
                                start=True, stop=True,
                            )
                            rec64 = rpool.tile([D, QW], dt.float32, tag="rec64")
                            nc.vector.tensor_copy(rec64, bc)
                            osb = opool_sb.tile([D, QW], dt.float32, tag="osb")
                            nc.vector.tensor_mul(osb, opsum[b][qi][:D, :], rec64)
                            nc.sync.dma_start(
                                outT[b, h, :, q0 + qi * QW:q0 + (qi + 1) * QW], osb
                            )
    nc.finalize()
    return nc


def make_in_maps(q, k, v, mask, attn_bias):
    scale = np.float32(D ** -0.5)
    qTf = (q.transpose(0, 1, 3, 2) * scale).astype(np.float16)   # [B,H,D,N]
    kTf = k.transpose(0, 1, 3, 2).astype(np.float16)
    vA = np.concatenate(
        [v, np.ones((B, H, N, 1), np.float32)], axis=-1
    ).astype(np.float16)                                         # [B,H,N,D+1]
    maskT = np.ascontiguousarray(
        mask[:, 0].transpose(0, 2, 1)
    ).astype(np.float16)                                         # [B,N,N] 0/1
    biasT32 = np.ascontiguousarray(
        attn_bias[0].transpose(0, 2, 1)
    )                                                            # [H,N,N] f32
    biasT = biasT32.astype(np.float16)
    expbT = np.exp(biasT32, dtype=np.float32).astype(np.float16)  # [H,N,N]
    ident = np.eye(P, dtype=np.float16)
    ones = np.ones((1, D), np.float16)

    in_maps = []
    for c in range(8):
        bp, hq = divmod(c, 4)
        bs, hs = 2 * bp, 4 * hq
        # pack the two batches along the partition dim: [NH, 2D, N]
        qpack = np.concatenate(
            [qTf[bs + b, hs:hs + NH] for b in range(NB)], axis=1
        )  # [NH, 2D, N]
        kpack = np.concatenate(
            [kTf[bs + b, hs:hs + NH] for b in range(NB)], axis=1
        )
        in_maps.append({
            "qT": np.ascontiguousarray(qpack),
            "kT": np.ascontiguousarray(kpack),
            "vA": np.ascontiguousarray(vA[bs:bs + NB, hs:hs + NH]),
            "maskT": np.ascontiguousarray(maskT[bs:bs + 1]),
            "mex1": np.ascontiguousarray(
                maskT[bs + 1][None] * expbT[hs:hs + NH]),
            "biasT": np.ascontiguousarray(biasT[hs:hs + NH]),
            "ident": ident,
            "ones": ones,
        })
    return in_maps


def kernel(q, k, v, mask, attn_bias):
    if "nc" not in _CACHE:
        _CACHE["nc"] = build_bass()
    nc = _CACHE["nc"]
    in_maps = make_in_maps(
        np.asarray(q, np.float32), np.asarray(k, np.float32),
        np.asarray(v, np.float32), np.asarray(mask, bool),
        np.asarray(attn_bias, np.float32),
    )
    rr = run_bass_kernel_spmd(
        nc, in_maps, list(range(8)), trace=TRACE,
        tmpdir=_CACHE.get("tmpdir"),
    )
    _CACHE["last_result"] = rr

    out = np.empty((B, H, N, D), np.float32)
    for c in range(8):
        bp, hq = divmod(c, 4)
        bs, hs = 2 * bp, 4 * hq
        oT = np.asarray(rr.results[c]["outT"])    # [NB,NH,D,N]
        out[bs:bs + NB, hs:hs + NH] = oT.transpose(0, 1, 3, 2)
    return out
